# revision 8
# baseline (speedup 1.0000x reference)
"""Trainium2 Bass kernel: capsule agreement routing (moe_routing).

Problem: preds [B=8, O=32, H=14, W=14, I=32, D=16] fp32, b (routing logit
param, zeros) [1,O,H,W,I].  3 rounds of dynamic routing; output v [B,O,H,W,D].

Sharding: data-parallel over batch; core k gets preds[k] -> [6272, 512]
(sites x (i,d)).  Routing is fully local per site, so there are no
collectives; the host stacks the 8 per-core outputs.

Kernel layout per core: 49 groups of 128 sites; partition dim = site-within-
group, free dim = (i,d) = 512.  Processed in 7 chunks of 7 groups, each chunk
runs the full routing (load -> iter0 -> 3 iterations -> store) so chunks
pipeline across engines.

Math notes:
 - softmax over I skips max-subtraction (logits stay |b| < ~20, exp is
   fp32-safe) and stays unnormalized: u' = sum_i e_i p_i, with 1/sum(e)
   folded into the squash scale.
 - squash(u) = sq/(1+sq)/sqrt(sq+eps) * u with sq = |u|^2; all per-site
   scalars are [128, G] tensors combined via broadcast (step-0) APs.
"""

import os
import sys

import numpy as np

sys.path.insert(0, "/opt/trn_rl_repo")

from contextlib import ExitStack

import concourse.bacc as bacc
import concourse.mybir as mybir
import concourse.tile as tile
from concourse.bass_utils import run_bass_kernel_spmd

F32 = mybir.dt.float32
AX = mybir.AxisListType
ALU = mybir.AluOpType
ACTF = mybir.ActivationFunctionType

B, O, H, W, I, D = 8, 32, 14, 14, 32, 16
S = O * H * W          # 6272 sites per core
PGRP = 128             # sites per group (partition dim)
J = S // PGRP          # 49 groups
G = 7                  # groups per chunk
NCH = J // G           # 7 chunks
PF = G * I * D         # 3584 free elems of preds per chunk
LF = G * I             # 224  logit elems per chunk
UF = G * D             # 112  capsule-vec elems per chunk
EPS = 1e-7
NITER = 3
NCORES = 8


def _squash(nc, spool, u, rse, it, epsb):
    """v = squash(u * rse) computed as gam (x) u.

    u: [128, UF] unnormalized capsule sums per (group, d).
    rse: [128, G] = 1/sum(exp) per group, or None for iter 0 (c uniform,
         rse == 1/I exactly).
    Returns v tile [128, UF].
    """
    usq = spool.tile([128, UF], F32, tag="usq")
    nc.scalar.activation(usq[:], u[:], ACTF.Square)
    ssq = spool.tile([128, G], F32, tag="ssq")
    nc.vector.reduce_sum(
        ssq[:], usq.rearrange("p (g d) -> p g d", d=D), axis=AX.X
    )
    sq = spool.tile([128, G], F32, tag="sq")
    if rse is None:
        nc.vector.tensor_scalar_mul(sq[:], ssq[:], 1.0 / (I * I))
    else:
        q1 = spool.tile([128, G], F32, tag="q1")
        nc.gpsimd.tensor_mul(q1[:], ssq[:], rse[:])
        nc.gpsimd.tensor_mul(sq[:], q1[:], rse[:])
    s1 = spool.tile([128, G], F32, tag="s1")
    nc.scalar.activation(s1[:], sq[:], ACTF.Sqrt, bias=epsb[:, 0:1])
    d1 = spool.tile([128, G], F32, tag="d1")
    nc.vector.scalar_tensor_tensor(
        d1[:], sq[:], 1.0, s1[:], op0=ALU.add, op1=ALU.mult
    )
    r1 = spool.tile([128, G], F32, tag="r1")
    nc.vector.reciprocal(r1[:], d1[:])
    gam = spool.tile([128, G], F32, tag="gam")
    if rse is None:
        q2 = spool.tile([128, G], F32, tag="q2")
        nc.gpsimd.tensor_mul(q2[:], sq[:], r1[:])
        nc.vector.tensor_scalar_mul(gam[:], q2[:], 1.0 / I)
    else:
        q2 = spool.tile([128, G], F32, tag="q2")
        nc.gpsimd.tensor_mul(q2[:], sq[:], r1[:])
        nc.gpsimd.tensor_mul(gam[:], q2[:], rse[:])
    v = spool.tile([128, UF], F32, tag=f"v{it % 2}")
    gb = gam.unsqueeze(2).to_broadcast((128, G, D))
    nc.gpsimd.tensor_tensor(
        v.rearrange("p (g d) -> p g d", d=D), u.rearrange("p (g d) -> p g d", d=D), gb, op=ALU.mult
    )
    return v


def _build_program():
    nc = bacc.Bacc(
        "TRN2", target_bir_lowering=False, debug=False, num_devices=NCORES
    )
    pd = nc.dram_tensor("preds", [S, I * D], F32, kind="ExternalInput").ap()
    vo = nc.dram_tensor("v_out", [S, D], F32, kind="ExternalOutput").ap()
    # DRAM views with partition-major iteration order to match SBUF tiles
    pdv = pd.rearrange("(j p) f -> p j f", p=PGRP)   # [128, 49, 512]
    vov = vo.rearrange("(j p) d -> p j d", p=PGRP)   # [128, 49, 16]

    with tile.TileContext(nc) as tc, ExitStack() as ctx:
        ppool = ctx.enter_context(tc.tile_pool(name="ppool", bufs=2))
        tpool = ctx.enter_context(tc.tile_pool(name="tpool", bufs=3))
        spool = ctx.enter_context(tc.tile_pool(name="spool", bufs=3))
        cpool = ctx.enter_context(tc.tile_pool(name="cpool", bufs=1))

        epsb = cpool.tile([128, 1], F32, tag="eps")
        nc.gpsimd.memset(epsb[:], EPS)

        for ch in range(NCH):
            g0 = ch * G
            P = ppool.tile([128, PF], F32, tag="P")
            nc.sync.dma_start(P[:], pdv[:, g0 : g0 + G, :])
            Pv = P

            # ---- iter 0: c uniform -> u = sum_i preds / I (scale folded)
            u = spool.tile([128, UF], F32, tag="u")
            nc.vector.reduce_sum(
                u[:],
                Pv.rearrange("p (g i d) -> p g d i", i=I, d=D),
                axis=AX.X,
            )
            v = _squash(nc, spool, u, None, 0, epsb)

            blog = None
            for it in range(NITER):
                # ---- agreement: a[s,i] = sum_d P * v  (v broadcast over i)
                t = tpool.tile([128, PF], F32, tag="t")
                vb = (
                    v
                    .rearrange("p (g d) -> p g d", d=D)
                    .unsqueeze(2)
                    .to_broadcast((128, G, I, D))
                )
                nc.vector.tensor_tensor(
                    t.rearrange("p (g i d) -> p g i d", i=I, d=D),
                    Pv.rearrange("p (g i d) -> p g i d", i=I, d=D),
                    vb,
                    op=ALU.mult,
                )
                newlog = spool.tile([128, LF], F32, tag=f"blog{it % 2}")
                if blog is None:
                    nc.vector.reduce_sum(
                        newlog[:],
                        t.rearrange("p (gi d) -> p gi d", d=D),
                        axis=AX.X,
                    )
                else:
                    a = spool.tile([128, LF], F32, tag="a")
                    nc.vector.reduce_sum(
                        a[:],
                        t.rearrange("p (gi d) -> p gi d", d=D),
                        axis=AX.X,
                    )
                    nc.gpsimd.tensor_add(newlog[:], a[:], blog[:])
                blog = newlog

                # ---- softmax over i (unnormalized; no max subtraction)
                e = spool.tile([128, LF], F32, tag="e")
                nc.scalar.activation(e[:], blog[:], ACTF.Exp)
                se = spool.tile([128, G], F32, tag="se")
                nc.vector.reduce_sum(
                    se[:], e.rearrange("p (g i) -> p g i", i=I), axis=AX.X
                )
                rse = spool.tile([128, G], F32, tag="rse")
                nc.vector.reciprocal(rse[:], se[:])

                # ---- weighted vote: u'[s,d] = sum_i e * P (e broadcast over d)
                t2 = tpool.tile([128, PF], F32, tag="t")
                eb = (
                    e
                    .rearrange("p (g i) -> p g i", i=I)
                    .unsqueeze(3)
                    .to_broadcast((128, G, I, D))
                )
                nc.vector.tensor_tensor(
                    t2.rearrange("p (g i d) -> p g i d", i=I, d=D),
                    Pv.rearrange("p (g i d) -> p g i d", i=I, d=D),
                    eb,
                    op=ALU.mult,
                )
                u = spool.tile([128, UF], F32, tag="u")
                nc.vector.reduce_sum(
                    u[:],
                    t2.rearrange("p (g i d) -> p g d i", i=I, d=D),
                    axis=AX.X,
                )
                v = _squash(nc, spool, u, rse, it + 1, epsb)

            nc.sync.dma_start(
                vov[:, g0 : g0 + G, :],
                v.rearrange("p (g d) -> p g d", d=D),
            )

    nc.compile()
    return nc


_NC = None


def _get_program():
    global _NC
    if _NC is None:
        _NC = _build_program()
    return _NC


def _numpy_routing(preds, b):
    """Pure-numpy fallback replicating the jax reference (general b)."""
    preds = preds.astype(np.float32)  # [B,O,H,W,I,D]
    b = np.broadcast_to(b.astype(np.float32), (1,) + preds.shape[1:5])

    def softmax(x, axis):
        m = np.max(x, axis=axis, keepdims=True)
        e = np.exp(x - m)
        return e / np.sum(e, axis=axis, keepdims=True)

    def squash(s):
        sq = np.sum(s * s, axis=-1)
        safe = np.sqrt(sq + EPS)
        factor = sq / (1.0 + sq)
        return (factor / safe)[..., None] * s

    c = softmax(b, axis=-1)
    v = squash(np.sum(c[..., None] * preds, axis=-2))
    bb = b
    for _ in range(NITER):
        bb = bb + np.sum(preds * v[..., None, :], axis=-1)
        c = softmax(bb, axis=-1)
        v = squash(np.sum(preds * c[..., None], axis=-2))
    return v


def kernel(tensor_of_prediction_vector, b):
    preds = np.ascontiguousarray(
        np.asarray(tensor_of_prediction_vector, dtype=np.float32)
    )
    bb = np.asarray(b, dtype=np.float32)
    if bb.size and np.any(bb != 0.0):
        # Routing-logit param is nonzero: take the straightforward host path.
        return _numpy_routing(preds, bb)

    nc = _get_program()
    in_maps = [
        {"preds": preds[k].reshape(S, I * D)} for k in range(NCORES)
    ]
    res = run_bass_kernel_spmd(nc, in_maps, list(range(NCORES)))
    out = np.stack(
        [res.results[k]["v_out"].reshape(O, H, W, D) for k in range(NCORES)]
    )
    return out


if __name__ == "__main__":
    rng = np.random.default_rng(0)
    preds = rng.standard_normal((B, O, H, W, I, D), dtype=np.float32)
    b0 = np.zeros((1, O, H, W, I), np.float32)
    got = kernel(preds, b0)
    want = _numpy_routing(preds, b0)
    err = np.abs(got - want).max() / np.abs(want).max()
    print("rel err vs numpy:", err)
